# revision 30
# baseline (speedup 1.0000x reference)
"""Trainium2 Bass kernel for nn_BHLinear: x -> D0 -> FWHT/64 -> D1 -> FWHT/64 -> final_B.

Math (per row, f = 12-bit feature index = 64*u + v, u = 2m+j):
  FWHT_4096 = H64(u) (x) H64(v).  H64(v) folds into the adjacent block matmuls
  (C1 = H64@B1/64 per-u; G = H-half@final_B/64 per-out-block).  The remaining
  H64(u) = H2(j) (x) H32(m): H2 folds into the block-diag passes, H32 runs as
  dedicated fixed-weight passes (I4 (x) H32) with m brought onto partitions by
  DVE 32x32 stream transposes of fp16 pairs viewed as int32.

Five TensorE passes (P1 per-m, P2 fixed I4xH32, P3 per-m', P4 fixed, P5 per-W
diag(G,G)), each PSUM-drained; DVE transposes bridge passes.

v5b flow design (drains+transposes are the ~212us/core fp32-PSUM floor):
  - Every bridge transpose is split into two rh-half StreamTransposes
    (FD=2048 int32 each).  P1/P3 PSUM groups are (rh-half, m-half) so a
    transpose half only needs 2 of the 4 drains; P2/P4 groups are rh-slices
    already.
  - VectorE's per-bridge queue is [drain g3, T.h1, T.h0]: it drains the last
    group itself and transposes immediately -- no cross-engine wait on the
    critical edge.  ScalarE drains g0-g2 (and 3.5/4 of P5's groups).
  - P2/P4 matmuls consume the rhH half first (fresh from T.h1) so they start
    one transpose-half earlier.
  - Diagonal skew (stage s of tile t at step s+t, deepest stage first) keeps
    ~3 tiles in flight in every engine queue.

v4c layout (kept): row r = (rh, rl:2) bits; internal SBUF y-layouts are
(rh, blk, rl) so every StreamTranspose has its 32-wide axis at 8-byte stride
(full DVE rate) and per-block matmul rhs for P1/P3/P5 is 8-byte runs at 256B
stride.  Output is fp16 (host casts to fp32).

Sharding: rows (4*4096=16384) split contiguously across 8 cores (2048 each).
Host packs x into [T=8, 128, 32*R] fp16 (p=(j,v), col=m*R+r) and unpacks
out [T, 128, 32*R] fp16 (p=(j'',o), col=W*R+r, f=64*(2W+j'')+o).
"""

import os

import numpy as np

# ---- static config ---------------------------------------------------------
# Schedule-tuning knobs (env-overridable for simulator sweeps; defaults are
# the shipped configuration).
_VD_DEFAULT = "37" if os.environ.get("K_R", "256") == "256" else "13"
_VD_BLOCKED = tuple(int(c) for c in os.environ.get("K_VD_BLOCKED", _VD_DEFAULT))
_VD_FIXED = tuple(int(c) for c in os.environ.get("K_VD_FIXED", _VD_DEFAULT))
_VD_P5 = tuple(int(c) for c in os.environ.get("K_VD_P5", _VD_DEFAULT))
_XT_BUFS = int(os.environ.get("K_XT_BUFS", "6"))
_YRAW_BUFS = int(os.environ.get("K_YRAW_BUFS", "4"))
_YT_BUFS = int(os.environ.get("K_YT_BUFS", "5"))
_OUT_BUFS = int(os.environ.get("K_OUT_BUFS", "2"))
_SHALLOW_FIRST = os.environ.get("K_SHALLOW_FIRST", "0") == "1"
_P5_ODD_ONLY = os.environ.get("K_P5_ODD", "1") == "1"
_XQ = int(os.environ.get("K_XQ", "4"))          # x-load chunks per tile (2 or 4)
_OUT_COLS = int(os.environ.get("K_OUT_COLS", "2048"))

NCORES = 8
R = int(os.environ.get("K_R", "256"))   # rows per tile
T = 2048 // R            # tiles per core
ROWS_PER_CORE = R * T    # 2048
D = 4096
TOTAL_ROWS = NCORES * ROWS_PER_CORE  # 16384

_F16 = np.float16


def _hadamard(n):
    H = np.array([[1.0]], dtype=np.float64)
    while H.shape[0] < n:
        H = np.block([[H, H], [H, -H]])
    return H


_H2 = _hadamard(2)
_H32 = _hadamard(32)
_H64 = _hadamard(64)


def _build_weights(inner_B, final_B):
    """w1/w3/w5 [128,4096] fp16 (32 lhsT blocks side by side), w2 [128,128]."""
    B0 = inner_B[0].astype(np.float64)
    B1 = inner_B[1].astype(np.float64)
    fB = final_B.astype(np.float64)

    C1 = np.einsum('vk,ukt->uvt', _H64, B1) / 64.0
    G = np.zeros((64, 64, 64))
    for u in range(64):
        for h in range(2):
            G[u][:, 32 * h:32 * h + 32] = _H64[:, 32 * h:32 * h + 32] @ fB[2 * u + h] / 64.0

    w1 = np.zeros((128, 32, 128))
    w3 = np.zeros((128, 32, 128))
    w5 = np.zeros((128, 32, 128))
    for m in range(32):
        for j in range(2):
            for jp in range(2):
                w1[j * 64:(j + 1) * 64, m, jp * 64:(jp + 1) * 64] = _H2[j, jp] * B0[2 * m + j]
                w3[j * 64:(j + 1) * 64, m, jp * 64:(jp + 1) * 64] = _H2[j, jp] * C1[2 * m + j]
        for jpp in range(2):
            w5[jpp * 64:(jpp + 1) * 64, m, jpp * 64:(jpp + 1) * 64] = G[2 * m + jpp]
    w1 = w1.reshape(128, 4096)
    w3 = w3.reshape(128, 4096)
    w5 = w5.reshape(128, 4096)
    w2 = np.kron(np.eye(4), _H32)      # partitions (X4, m)
    return (w1.astype(_F16), w2.astype(_F16), w3.astype(_F16), w5.astype(_F16))


def _pack_x(x):
    """x [..., 4096] fp32 -> list of per-core arrays [T, 128, 32*R] fp16."""
    xf = np.ascontiguousarray(x.reshape(-1, D))
    assert xf.shape[0] == TOTAL_ROWS
    x6 = xf.reshape(NCORES, T, R, 32, 2, 64)       # core,t,r,m,j,v
    x6 = x6.transpose(0, 1, 4, 5, 3, 2)            # core,t,j,v,m,r
    x6 = np.ascontiguousarray(x6).reshape(NCORES, T, 128, 32 * R)
    return [np.ascontiguousarray(x6[c]).astype(_F16) for c in range(NCORES)]


def _unpack_out(outs, orig_shape):
    """outs: list of per-core [T, 128, 32*R] fp16 -> [*orig_shape[:-1], 4096] fp32."""
    o = np.stack(outs, axis=0).astype(np.float32)  # [core, T, 128, 32R]
    o = o.reshape(NCORES, T, 2, 64, 32, R)         # core,t,j'',o,W,r
    o = o.transpose(0, 1, 5, 4, 2, 3)              # core,t,r,W,j'',o
    o = np.ascontiguousarray(o).reshape(TOTAL_ROWS, D)
    return o.reshape(*orig_shape[:-1], D)


# ---- bass program ----------------------------------------------------------
_PROGRAM = None


def _build_program():
    global _PROGRAM
    if _PROGRAM is not None:
        return _PROGRAM
    from contextlib import ExitStack
    import concourse.tile as tile
    from concourse import bacc, mybir

    f32 = mybir.dt.float32
    f16 = mybir.dt.float16
    i32 = mybir.dt.int32

    nc = bacc.Bacc()
    x_d = nc.declare_dram_parameter("x", [T, 128, 32 * R], f16, isOutput=False)
    w1_d = nc.declare_dram_parameter("w1", [128, 4096], f16, isOutput=False)
    w2_d = nc.declare_dram_parameter("w2", [128, 128], f16, isOutput=False)
    w3_d = nc.declare_dram_parameter("w3", [128, 4096], f16, isOutput=False)
    w5_d = nc.declare_dram_parameter("w5", [128, 4096], f16, isOutput=False)
    out_d = nc.declare_dram_parameter("out", [T, 128, 32 * R], f16, isOutput=True)

    C = 32 * R          # 8192 cols per tile
    HC = C // 2         # 4096: x half-tile DMA granularity

    with tile.TileContext(nc) as tc, ExitStack() as ctx:
        wpool = ctx.enter_context(tc.tile_pool(name="weights", bufs=1))
        xt_pool = ctx.enter_context(tc.tile_pool(name="xt", bufs=_XT_BUFS))
        yraw_pool = ctx.enter_context(tc.tile_pool(name="yraw", bufs=_YRAW_BUFS))
        yt_pool = ctx.enter_context(tc.tile_pool(name="yt", bufs=_YT_BUFS))
        out_pool = ctx.enter_context(tc.tile_pool(name="outp", bufs=_OUT_BUFS))
        psum = ctx.enter_context(tc.tile_pool(name="ps", bufs=4, space="PSUM"))

        w1_sb = wpool.tile([128, 4096], f16)
        w2_sb = wpool.tile([128, 128], f16)
        w3_sb = wpool.tile([128, 4096], f16)
        w5_sb = wpool.tile([128, 4096], f16)
        nc.sync.dma_start(w1_sb[:], w1_d[:])
        nc.sync.dma_start(w2_sb[:], w2_d[:])
        nc.sync.dma_start(w3_sb[:], w3_d[:])
        nc.sync.dma_start(w5_sb[:], w5_d[:])

        sc = nc.scalar.copy
        vc = nc.vector.tensor_copy

        def pair_T(dst, src):
            """DVE 32x32 transpose of fp16 pairs (int32 view): swap partition
            low-5 bits with the 5-bit block index.  Layouts are (rh, blk, rl)
            with rl = 4 fp16 = 2 int32, so the 32-wide axis sits at 8-byte
            stride (full DVE rate)."""
            in_v = src[:].bitcast(i32).rearrange("p (rh m rl) -> p rh rl m", m=32, rl=2)
            out_v = dst[:].bitcast(i32).rearrange("p (rh tl rl) -> p rh rl tl", tl=32, rl=2)
            nc.vector.transpose(out_v, in_v)

        # 8 PSUM groups of 1024 fp32 (2 banks) per pass, pool bufs=4: four
        # matmul groups in flight, so TensorE + drains pipeline across passes
        # instead of forming one global drain-paced chain.

        def emit_blocked_pass(src_fn, w_sb, dst):
            """Per-block lhsT pass (P1/P3): PSUM group g holds 4 blocks of 256
            cols in (i, rh, rl) order; drain writes dst in (rh, m, rl) layout
            via strided APs."""
            dst_v = dst[:].rearrange("p (rh m rl) -> p rh m rl", m=32, rl=4)
            bpg = 1024 // R   # blocks per PSUM group
            for g in range(C // 1024):
                ps = psum.tile([128, 1024], f32, tag="ps")
                for i in range(bpg):
                    b = bpg * g + i
                    nc.tensor.matmul(
                        ps[:, i * R:(i + 1) * R],
                        w_sb[:, b * 128:(b + 1) * 128],
                        src_fn(b),
                        start=True, stop=True,
                    )
                ps_v = ps[:].rearrange("p (m rh rl) -> p rh m rl", m=bpg, rl=4)
                eng = vc if g in _VD_BLOCKED else sc
                eng(dst_v[:, :, bpg * g:bpg * (g + 1), :], ps_v)

        def emit_fixed_pass(src, dst, w_sb):
            """Fixed-weight pass (P2/P4): contiguous sweep with N=512
            matmuls, contiguous drains."""
            for g in range(C // 1024):
                ps = psum.tile([128, 1024], f32, tag="ps")
                for i in range(2):
                    k = 2 * g + i
                    nc.tensor.matmul(
                        ps[:, i * 512:(i + 1) * 512],
                        w_sb[:],
                        src[:, k * 512:(k + 1) * 512],
                        start=True, stop=True,
                    )
                eng = vc if g in _VD_FIXED else sc
                eng(dst[:, g * 1024:(g + 1) * 1024], ps[:])

        def emit_p5(y4t, t):
            """P5: per-block W lhsT, rhs = 8B-run slices of y4t (rh, W, rl);
            PSUM (W-quad, rh, rl) drains contiguously to fp16 out (col = W*R+r).
            Out DMAs are batched: one 1MiB store per 4 drained groups."""
            y4t_v = y4t[:].rearrange("p (rh m rl) -> p m rh rl", m=32, rl=4)
            wpg = 1024 // R   # W-blocks per PSUM group
            OC = min(C, _OUT_COLS)
            for half in range(max(1, C // OC)):
                out_sb = out_pool.tile([128, OC], f16, tag="outp")
                for gq in range(OC // 1024):
                    g = (OC // 1024) * half + gq
                    ps = psum.tile([128, 1024], f32, tag="ps")
                    for i in range(wpg):
                        W = wpg * g + i
                        nc.tensor.matmul(
                            ps[:, i * R:(i + 1) * R],
                            w5_sb[:, W * 128:(W + 1) * 128],
                            y4t_v[:, W],
                            start=True, stop=True,
                        )
                    eng = vc if (g in _VD_P5 and (t % 2 == 1 or not _P5_ODD_ONLY)) else sc
                    eng(out_sb[:, gq * 1024:(gq + 1) * 1024], ps[:])
                nc.sync.dma_start(out_d[t][:, half * OC:(half + 1) * OC], out_sb[:])

        QC = C // _XQ       # cols per x-load chunk
        def load_x(t):
            xh = []
            for q in range(_XQ):
                xt = xt_pool.tile([128, QC], f16, tag="xt")
                nc.gpsimd.dma_start(xt[:], x_d[t][:, q * QC:(q + 1) * QC])
                xh.append(xt)
            return xh

        # ---- diagonal software pipeline -------------------------------
        S = [None] * T

        def stload(t):
            S[t] = {}
            S[t]['xh'] = load_x(t)

        def st0(t):
            S[t]['y1'] = yraw_pool.tile([128, C], f16, tag="yraw", name="y1")
            bph = QC // R   # blocks per x-load chunk
            emit_blocked_pass(
                lambda b, xh=S[t]['xh'], bph=bph:
                    xh[b // bph][:, (b % bph) * R:(b % bph + 1) * R],
                w1_sb, S[t]['y1'])

        def st1(t):
            S[t]['y1t'] = yt_pool.tile([128, C], f16, tag="yt", name="y1t")
            pair_T(S[t]['y1t'], S[t]['y1'])

        def st2(t):
            S[t]['y2'] = yraw_pool.tile([128, C], f16, tag="yraw", name="y2")
            emit_fixed_pass(S[t]['y1t'], S[t]['y2'], w2_sb)

        def st3(t):
            S[t]['y2t'] = yt_pool.tile([128, C], f16, tag="yt", name="y2t")
            pair_T(S[t]['y2t'], S[t]['y2'])

        def st4(t):
            S[t]['y3'] = yraw_pool.tile([128, C], f16, tag="yraw", name="y3")
            v = S[t]['y2t'][:].rearrange("p (rh m rl) -> p m rh rl", m=32, rl=4)
            emit_blocked_pass(lambda b, v=v: v[:, b], w3_sb, S[t]['y3'])

        def st5(t):
            S[t]['y3t'] = yt_pool.tile([128, C], f16, tag="yt", name="y3t")
            pair_T(S[t]['y3t'], S[t]['y3'])

        def st6(t):
            S[t]['y4'] = yraw_pool.tile([128, C], f16, tag="yraw", name="y4")
            emit_fixed_pass(S[t]['y3t'], S[t]['y4'], w2_sb)

        def st7(t):
            S[t]['y4t'] = yt_pool.tile([128, C], f16, tag="yt", name="y4t")
            pair_T(S[t]['y4t'], S[t]['y4'])

        def st8(t):
            emit_p5(S[t]['y4t'], t)
            S[t] = None

        stages = [stload, st0, st1, st2, st3, st4, st5, st6, st7, st8]
        NS = len(stages)
        for step in range(T + NS - 1):
            trange = range(T - 1, -1, -1) if _SHALLOW_FIRST else range(T)
            for t in trange:
                s = step - t
                if 0 <= s < NS:
                    stages[s](t)

    nc.finalize()
    _PROGRAM = nc
    return nc


_LAST_RESULTS = None


def build_for_profile(x, inner_B, final_B):
    """Return (nc, in_maps) for external profiling harnesses."""
    w1, w2, w3, w5 = _build_weights(np.asarray(inner_B), np.asarray(final_B))
    x_packed = _pack_x(np.asarray(x, dtype=np.float32))
    nc = _build_program()
    in_maps = [
        {"x": x_packed[c], "w1": w1, "w2": w2, "w3": w3, "w5": w5}
        for c in range(NCORES)
    ]
    return nc, in_maps


def kernel(x, inner_B, final_B, _trace=False):
    global _LAST_RESULTS
    from concourse.bass_utils import run_bass_kernel_spmd

    orig_shape = x.shape
    nc, in_maps = build_for_profile(x, inner_B, final_B)
    try:
        res = run_bass_kernel_spmd(nc, in_maps, list(range(NCORES)))
    except Exception:
        # transient NRT device errors have been observed; retry once
        res = run_bass_kernel_spmd(nc, in_maps, list(range(NCORES)))
    _LAST_RESULTS = res
    outs = [np.asarray(res.results[c]["out"]) for c in range(NCORES)]
    return _unpack_out(outs, orig_shape).astype(np.float32)


# revision 31
# speedup vs baseline: 1.0190x; 1.0190x over previous
"""Trainium2 Bass kernel for nn_BHLinear: x -> D0 -> FWHT/64 -> D1 -> FWHT/64 -> final_B.

Math (per row, f = 12-bit feature index = 64*u + v, u = 2m+j):
  FWHT_4096 = H64(u) (x) H64(v).  H64(v) folds into the adjacent block matmuls
  (C1 = H64@B1/64 per-u; G = H-half@final_B/64 per-out-block).  The remaining
  H64(u) = H2(j) (x) H32(m): H2 folds into the block-diag passes, H32 runs as
  dedicated fixed-weight passes (I4 (x) H32) with m brought onto partitions by
  DVE 32x32 stream transposes of fp16 pairs viewed as int32.

Five TensorE passes (P1 per-m, P2 fixed I4xH32, P3 per-m', P4 fixed, P5 per-W
diag(G,G)), each PSUM-drained; DVE transposes bridge passes.

v5h schedule (HW-measured 300.4us/core vs 374us baseline):
  - PSUM: 8 groups of 1024 fp32 (2 banks) per pass, pool bufs=4, so TensorE
    and the drains pipeline across passes instead of forming one global
    drain-paced chain (4-bank groups with bufs=2 serialize the whole kernel).
  - Drain split: VectorE takes groups 3 and 7 of each pass (always PSUM
    slot D -> clean bank separation from ScalarE, which HW cares about:
    spreading V-drains across slots measurably slows both engines' ops);
    ScalarE drains the rest.  ~252us S / ~224us V busy per core.
  - DMA queues: x-loads prefetched one pipeline step ahead on the GpSimd
    queue; weights + batched out-stores on the Sync queue, so store waits
    never block load prefetch (single in-order queue head-of-line).
  - SBUF pools sized for pipeline slack: the 4 transposed y-buffers are the
    live set at full depth, so yt gets 5 bufs (the 5th removed six ~4us
    mid-run ScalarE stalls); x arrives in 2KB/partition quarters (6 bufs).
  - Diagonal skew: stage s of tile t emitted at step s+t, deepest stage
    first, so every engine's in-order queue interleaves ~3 tiles.

Layout (v4c, kept): row r = (rh, rl:2) bits; internal SBUF y-layouts are
(rh, blk, rl) so every StreamTranspose has its 32-wide axis at 8-byte stride
(full DVE rate), per-block matmul rhs for P1/P3/P5 is 8-byte runs at 256B
stride, P2/P4 rhs sweeps and their drains are fully contiguous.  Drains of
P1/P3 write (rh, m, rl) from the (m, rh, rl)-ordered PSUM groups via strided
APs.  Output is fp16 (host casts to fp32), halving store-side HBM traffic.

TRN2 constraints that shaped this: matmul output must be fp32 in PSUM
(16-bit PSUM is TRN3-only), StreamTranspose requires same src/dst dtype
(no cast-in-transpose, so no PSUM-read transpose fusion), GpSimd and DMA
have no PSUM port -- drains can only run on ScalarE/VectorE.

Sharding: rows (4*4096=16384) split contiguously across 8 cores (2048 each).
Host packs x into [T=8, 128, 32*R] fp16 (p=(j,v), col=m*R+r) and unpacks
out [T, 128, 32*R] fp16 (p=(j'',o), col=W*R+r, f=64*(2W+j'')+o).
"""

import os

import numpy as np

# ---- static config ---------------------------------------------------------
# Schedule-tuning knobs (env-overridable for simulator sweeps; defaults are
# the shipped configuration).
_VD_DEFAULT = "37" if os.environ.get("K_R", "256") == "256" else "13"
_VD_BLOCKED = tuple(int(c) for c in os.environ.get("K_VD_BLOCKED", _VD_DEFAULT))
_VD_FIXED = tuple(int(c) for c in os.environ.get("K_VD_FIXED", _VD_DEFAULT))
_VD_P5 = tuple(int(c) for c in os.environ.get("K_VD_P5", _VD_DEFAULT))
_XT_BUFS = int(os.environ.get("K_XT_BUFS", "6"))
_YRAW_BUFS = int(os.environ.get("K_YRAW_BUFS", "4"))
_YT_BUFS = int(os.environ.get("K_YT_BUFS", "5"))
_OUT_BUFS = int(os.environ.get("K_OUT_BUFS", "2"))
_SHALLOW_FIRST = os.environ.get("K_SHALLOW_FIRST", "0") == "1"
_P5_ODD_ONLY = os.environ.get("K_P5_ODD", "1") == "1"
_XQ = int(os.environ.get("K_XQ", "4"))          # x-load chunks per tile (2 or 4)
_OUT_COLS = int(os.environ.get("K_OUT_COLS", "2048"))

NCORES = 8
R = int(os.environ.get("K_R", "256"))   # rows per tile
T = 2048 // R            # tiles per core
ROWS_PER_CORE = R * T    # 2048
D = 4096
TOTAL_ROWS = NCORES * ROWS_PER_CORE  # 16384

_F16 = np.float16


def _hadamard(n):
    H = np.array([[1.0]], dtype=np.float64)
    while H.shape[0] < n:
        H = np.block([[H, H], [H, -H]])
    return H


_H2 = _hadamard(2)
_H32 = _hadamard(32)
_H64 = _hadamard(64)


def _build_weights(inner_B, final_B):
    """w1/w3/w5 [128,4096] fp16 (32 lhsT blocks side by side), w2 [128,128]."""
    B0 = inner_B[0].astype(np.float64)
    B1 = inner_B[1].astype(np.float64)
    fB = final_B.astype(np.float64)

    C1 = np.einsum('vk,ukt->uvt', _H64, B1) / 64.0
    G = np.zeros((64, 64, 64))
    for u in range(64):
        for h in range(2):
            G[u][:, 32 * h:32 * h + 32] = _H64[:, 32 * h:32 * h + 32] @ fB[2 * u + h] / 64.0

    w1 = np.zeros((128, 32, 128))
    w3 = np.zeros((128, 32, 128))
    w5 = np.zeros((128, 32, 128))
    for m in range(32):
        for j in range(2):
            for jp in range(2):
                w1[j * 64:(j + 1) * 64, m, jp * 64:(jp + 1) * 64] = _H2[j, jp] * B0[2 * m + j]
                w3[j * 64:(j + 1) * 64, m, jp * 64:(jp + 1) * 64] = _H2[j, jp] * C1[2 * m + j]
        for jpp in range(2):
            w5[jpp * 64:(jpp + 1) * 64, m, jpp * 64:(jpp + 1) * 64] = G[2 * m + jpp]
    w1 = w1.reshape(128, 4096)
    w3 = w3.reshape(128, 4096)
    w5 = w5.reshape(128, 4096)
    w2 = np.kron(np.eye(4), _H32)      # partitions (X4, m)
    return (w1.astype(_F16), w2.astype(_F16), w3.astype(_F16), w5.astype(_F16))


def _pack_x(x):
    """x [..., 4096] fp32 -> list of per-core arrays [T, 128, 32*R] fp16."""
    xf = np.ascontiguousarray(x.reshape(-1, D))
    assert xf.shape[0] == TOTAL_ROWS
    x6 = xf.reshape(NCORES, T, R, 32, 2, 64)       # core,t,r,m,j,v
    x6 = x6.transpose(0, 1, 4, 5, 3, 2)            # core,t,j,v,m,r
    x6 = np.ascontiguousarray(x6).reshape(NCORES, T, 128, 32 * R)
    return [np.ascontiguousarray(x6[c]).astype(_F16) for c in range(NCORES)]


def _unpack_out(outs, orig_shape):
    """outs: list of per-core [T, 128, 32*R] fp16 -> [*orig_shape[:-1], 4096] fp32."""
    o = np.stack(outs, axis=0).astype(np.float32)  # [core, T, 128, 32R]
    o = o.reshape(NCORES, T, 2, 64, 32, R)         # core,t,j'',o,W,r
    o = o.transpose(0, 1, 5, 4, 2, 3)              # core,t,r,W,j'',o
    o = np.ascontiguousarray(o).reshape(TOTAL_ROWS, D)
    return o.reshape(*orig_shape[:-1], D)


# ---- bass program ----------------------------------------------------------
_PROGRAM = None


def _build_program():
    global _PROGRAM
    if _PROGRAM is not None:
        return _PROGRAM
    from contextlib import ExitStack
    import concourse.tile as tile
    from concourse import bacc, mybir

    f32 = mybir.dt.float32
    f16 = mybir.dt.float16
    i32 = mybir.dt.int32

    nc = bacc.Bacc()
    x_d = nc.declare_dram_parameter("x", [T, 128, 32 * R], f16, isOutput=False)
    w1_d = nc.declare_dram_parameter("w1", [128, 4096], f16, isOutput=False)
    w2_d = nc.declare_dram_parameter("w2", [128, 128], f16, isOutput=False)
    w3_d = nc.declare_dram_parameter("w3", [128, 4096], f16, isOutput=False)
    w5_d = nc.declare_dram_parameter("w5", [128, 4096], f16, isOutput=False)
    out_d = nc.declare_dram_parameter("out", [T, 128, 32 * R], f16, isOutput=True)

    C = 32 * R          # 8192 cols per tile
    HC = C // 2         # 4096: x half-tile DMA granularity

    with tile.TileContext(nc) as tc, ExitStack() as ctx:
        wpool = ctx.enter_context(tc.tile_pool(name="weights", bufs=1))
        xt_pool = ctx.enter_context(tc.tile_pool(name="xt", bufs=_XT_BUFS))
        yraw_pool = ctx.enter_context(tc.tile_pool(name="yraw", bufs=_YRAW_BUFS))
        yt_pool = ctx.enter_context(tc.tile_pool(name="yt", bufs=_YT_BUFS))
        out_pool = ctx.enter_context(tc.tile_pool(name="outp", bufs=_OUT_BUFS))
        psum = ctx.enter_context(tc.tile_pool(name="ps", bufs=4, space="PSUM"))

        w1_sb = wpool.tile([128, 4096], f16)
        w2_sb = wpool.tile([128, 128], f16)
        w3_sb = wpool.tile([128, 4096], f16)
        w5_sb = wpool.tile([128, 4096], f16)
        nc.sync.dma_start(w1_sb[:], w1_d[:])
        nc.sync.dma_start(w2_sb[:], w2_d[:])
        nc.sync.dma_start(w3_sb[:], w3_d[:])
        nc.sync.dma_start(w5_sb[:], w5_d[:])

        sc = nc.scalar.copy
        vc = nc.vector.tensor_copy

        def pair_T(dst, src):
            """DVE 32x32 transpose of fp16 pairs (int32 view): swap partition
            low-5 bits with the 5-bit block index.  Layouts are (rh, blk, rl)
            with rl = 4 fp16 = 2 int32, so the 32-wide axis sits at 8-byte
            stride (full DVE rate)."""
            in_v = src[:].bitcast(i32).rearrange("p (rh m rl) -> p rh rl m", m=32, rl=2)
            out_v = dst[:].bitcast(i32).rearrange("p (rh tl rl) -> p rh rl tl", tl=32, rl=2)
            nc.vector.transpose(out_v, in_v)

        # 8 PSUM groups of 1024 fp32 (2 banks) per pass, pool bufs=4: four
        # matmul groups in flight, so TensorE + drains pipeline across passes
        # instead of forming one global drain-paced chain.

        def emit_blocked_pass(src_fn, w_sb, dst):
            """Per-block lhsT pass (P1/P3): PSUM group g holds 4 blocks of 256
            cols in (i, rh, rl) order; drain writes dst in (rh, m, rl) layout
            via strided APs."""
            dst_v = dst[:].rearrange("p (rh m rl) -> p rh m rl", m=32, rl=4)
            bpg = 1024 // R   # blocks per PSUM group
            for g in range(C // 1024):
                ps = psum.tile([128, 1024], f32, tag="ps")
                for i in range(bpg):
                    b = bpg * g + i
                    nc.tensor.matmul(
                        ps[:, i * R:(i + 1) * R],
                        w_sb[:, b * 128:(b + 1) * 128],
                        src_fn(b),
                        start=True, stop=True,
                    )
                ps_v = ps[:].rearrange("p (m rh rl) -> p rh m rl", m=bpg, rl=4)
                eng = vc if g in _VD_BLOCKED else sc
                eng(dst_v[:, :, bpg * g:bpg * (g + 1), :], ps_v)

        def emit_fixed_pass(src, dst, w_sb):
            """Fixed-weight pass (P2/P4): contiguous sweep with N=512
            matmuls, contiguous drains."""
            for g in range(C // 1024):
                ps = psum.tile([128, 1024], f32, tag="ps")
                for i in range(2):
                    k = 2 * g + i
                    nc.tensor.matmul(
                        ps[:, i * 512:(i + 1) * 512],
                        w_sb[:],
                        src[:, k * 512:(k + 1) * 512],
                        start=True, stop=True,
                    )
                eng = vc if g in _VD_FIXED else sc
                eng(dst[:, g * 1024:(g + 1) * 1024], ps[:])

        def emit_p5(y4t, t):
            """P5: per-block W lhsT, rhs = 8B-run slices of y4t (rh, W, rl);
            PSUM (W-quad, rh, rl) drains contiguously to fp16 out (col = W*R+r).
            Out DMAs are batched: one 1MiB store per 4 drained groups."""
            y4t_v = y4t[:].rearrange("p (rh m rl) -> p m rh rl", m=32, rl=4)
            wpg = 1024 // R   # W-blocks per PSUM group
            OC = min(C, _OUT_COLS)
            for half in range(max(1, C // OC)):
                out_sb = out_pool.tile([128, OC], f16, tag="outp")
                for gq in range(OC // 1024):
                    g = (OC // 1024) * half + gq
                    ps = psum.tile([128, 1024], f32, tag="ps")
                    for i in range(wpg):
                        W = wpg * g + i
                        nc.tensor.matmul(
                            ps[:, i * R:(i + 1) * R],
                            w5_sb[:, W * 128:(W + 1) * 128],
                            y4t_v[:, W],
                            start=True, stop=True,
                        )
                    eng = vc if (g in _VD_P5 and (t % 2 == 1 or not _P5_ODD_ONLY)) else sc
                    eng(out_sb[:, gq * 1024:(gq + 1) * 1024], ps[:])
                nc.sync.dma_start(out_d[t][:, half * OC:(half + 1) * OC], out_sb[:])

        QC = C // _XQ       # cols per x-load chunk
        def load_x(t):
            xh = []
            for q in range(_XQ):
                xt = xt_pool.tile([128, QC], f16, tag="xt")
                nc.gpsimd.dma_start(xt[:], x_d[t][:, q * QC:(q + 1) * QC])
                xh.append(xt)
            return xh

        # ---- diagonal software pipeline -------------------------------
        S = [None] * T

        def stload(t):
            S[t] = {}
            S[t]['xh'] = load_x(t)

        def st0(t):
            S[t]['y1'] = yraw_pool.tile([128, C], f16, tag="yraw", name="y1")
            bph = QC // R   # blocks per x-load chunk
            emit_blocked_pass(
                lambda b, xh=S[t]['xh'], bph=bph:
                    xh[b // bph][:, (b % bph) * R:(b % bph + 1) * R],
                w1_sb, S[t]['y1'])

        def st1(t):
            S[t]['y1t'] = yt_pool.tile([128, C], f16, tag="yt", name="y1t")
            pair_T(S[t]['y1t'], S[t]['y1'])

        def st2(t):
            S[t]['y2'] = yraw_pool.tile([128, C], f16, tag="yraw", name="y2")
            emit_fixed_pass(S[t]['y1t'], S[t]['y2'], w2_sb)

        def st3(t):
            S[t]['y2t'] = yt_pool.tile([128, C], f16, tag="yt", name="y2t")
            pair_T(S[t]['y2t'], S[t]['y2'])

        def st4(t):
            S[t]['y3'] = yraw_pool.tile([128, C], f16, tag="yraw", name="y3")
            v = S[t]['y2t'][:].rearrange("p (rh m rl) -> p m rh rl", m=32, rl=4)
            emit_blocked_pass(lambda b, v=v: v[:, b], w3_sb, S[t]['y3'])

        def st5(t):
            S[t]['y3t'] = yt_pool.tile([128, C], f16, tag="yt", name="y3t")
            pair_T(S[t]['y3t'], S[t]['y3'])

        def st6(t):
            S[t]['y4'] = yraw_pool.tile([128, C], f16, tag="yraw", name="y4")
            emit_fixed_pass(S[t]['y3t'], S[t]['y4'], w2_sb)

        def st7(t):
            S[t]['y4t'] = yt_pool.tile([128, C], f16, tag="yt", name="y4t")
            pair_T(S[t]['y4t'], S[t]['y4'])

        def st8(t):
            emit_p5(S[t]['y4t'], t)
            S[t] = None

        stages = [stload, st0, st1, st2, st3, st4, st5, st6, st7, st8]
        NS = len(stages)
        for step in range(T + NS - 1):
            trange = range(T - 1, -1, -1) if _SHALLOW_FIRST else range(T)
            for t in trange:
                s = step - t
                if 0 <= s < NS:
                    stages[s](t)

    nc.finalize()
    _PROGRAM = nc
    return nc


_LAST_RESULTS = None


def build_for_profile(x, inner_B, final_B):
    """Return (nc, in_maps) for external profiling harnesses."""
    w1, w2, w3, w5 = _build_weights(np.asarray(inner_B), np.asarray(final_B))
    x_packed = _pack_x(np.asarray(x, dtype=np.float32))
    nc = _build_program()
    in_maps = [
        {"x": x_packed[c], "w1": w1, "w2": w2, "w3": w3, "w5": w5}
        for c in range(NCORES)
    ]
    return nc, in_maps


def kernel(x, inner_B, final_B, _trace=False):
    global _LAST_RESULTS
    from concourse.bass_utils import run_bass_kernel_spmd

    orig_shape = x.shape
    nc, in_maps = build_for_profile(x, inner_B, final_B)
    try:
        res = run_bass_kernel_spmd(nc, in_maps, list(range(NCORES)))
    except Exception:
        # transient NRT device errors have been observed; retry once
        res = run_bass_kernel_spmd(nc, in_maps, list(range(NCORES)))
    _LAST_RESULTS = res
    outs = [np.asarray(res.results[c]["out"]) for c in range(NCORES)]
    return _unpack_out(outs, orig_shape).astype(np.float32)


# revision 38
# speedup vs baseline: 1.0900x; 1.0697x over previous
"""Trainium2 Bass kernel for nn_BHLinear: x -> D0 -> FWHT/64 -> D1 -> FWHT/64 -> final_B.

Math (per row, f = 12-bit feature index = 64*u + v, u = 2m+j):
  FWHT_4096 = H64(u) (x) H64(v).  H64(v) folds into the adjacent block matmuls
  (C1 = H64@B1/64 per-u; G = H-half@final_B/64 per-out-block).  The remaining
  H64(u) = H2(j) (x) H32(m): H2 folds into the block-diag passes, H32 runs as
  dedicated fixed-weight passes (I4 (x) H32) with m brought onto partitions by
  DVE 32x32 stream transposes of fp16 pairs viewed as int32.

Five TensorE passes (P1 per-m, P2 fixed I4xH32, P3 per-m', P4 fixed, P5 per-W
diag(G,G)), each PSUM-drained; DVE transposes bridge passes.

v5h schedule (HW-measured 297.4-300.4us vs 374us baseline):
  - PSUM: 8 groups of 1024 fp32 (2 banks) per pass, pool bufs=4, so TensorE
    and the drains pipeline across passes instead of forming one global
    drain-paced chain (4-bank groups with bufs=2 serialize the whole kernel).
  - Drain split: VectorE takes groups 3 and 7 of each pass (always PSUM
    slot D -> clean bank separation from ScalarE, which HW cares about:
    spreading V-drains across slots measurably slows both engines' ops);
    ScalarE drains the rest.  ~252us S / ~224us V busy per core.
  - DMA queues: x-loads prefetched one pipeline step ahead on the GpSimd
    queue; weights + batched out-stores on the Sync queue, so store waits
    never block load prefetch (single in-order queue head-of-line).
  - SBUF pools sized for pipeline slack: the 4 transposed y-buffers are the
    live set at full depth, so yt gets 5 bufs (the 5th removed six ~4us
    mid-run ScalarE stalls); x arrives in 2KB/partition quarters (6 bufs).
  - Diagonal skew: stage s of tile t emitted at step s+t, deepest stage
    first, so every engine's in-order queue interleaves ~3 tiles.

Layout (v4c, kept): row r = (rh, rl:2) bits; internal SBUF y-layouts are
(rh, blk, rl) so every StreamTranspose has its 32-wide axis at 8-byte stride
(full DVE rate), per-block matmul rhs for P1/P3/P5 is 8-byte runs at 256B
stride, P2/P4 rhs sweeps and their drains are fully contiguous.  Drains of
P1/P3 write (rh, m, rl) from the (m, rh, rl)-ordered PSUM groups via strided
APs.  Output is fp16 (host casts to fp32), halving store-side HBM traffic.

TRN2 constraints that shaped this: matmul output must be fp32 in PSUM
(16-bit PSUM is TRN3-only), StreamTranspose requires same src/dst dtype
(no cast-in-transpose, so no PSUM-read transpose fusion), GpSimd and DMA
have no PSUM port -- drains can only run on ScalarE/VectorE.

Sharding: rows (4*4096=16384) split contiguously across 8 cores (2048 each).
Host packs x into [T=8, 128, 32*R] fp16 (p=(j,v), col=m*R+r) and unpacks
out [T, 128, 32*R] fp16 (p=(j'',o), col=W*R+r, f=64*(2W+j'')+o).
"""

import os

import numpy as np

# ---- static config ---------------------------------------------------------
# Schedule-tuning knobs (env-overridable for simulator sweeps; defaults are
# the shipped configuration).
_VD_DEFAULT = "37" if os.environ.get("K_R", "256") == "256" else "13"
_VD_BLOCKED = tuple(int(c) for c in os.environ.get("K_VD_BLOCKED", _VD_DEFAULT))
_VD_FIXED = tuple(int(c) for c in os.environ.get("K_VD_FIXED", _VD_DEFAULT))
_VD_P5 = tuple(int(c) for c in os.environ.get("K_VD_P5", _VD_DEFAULT))
_XT_BUFS = int(os.environ.get("K_XT_BUFS", "6"))
_YRAW_BUFS = int(os.environ.get("K_YRAW_BUFS", "4"))
_YT_BUFS = int(os.environ.get("K_YT_BUFS", "5"))
_OUT_BUFS = int(os.environ.get("K_OUT_BUFS", "2"))
_SHALLOW_FIRST = os.environ.get("K_SHALLOW_FIRST", "0") == "1"
_P5_ODD_ONLY = os.environ.get("K_P5_ODD", "1") == "1"
_XQ = int(os.environ.get("K_XQ", "4"))          # x-load chunks per tile (2 or 4)
_OUT_COLS = int(os.environ.get("K_OUT_COLS", "2048"))
_RAMPV = os.environ.get("K_RAMPV", "0") == "1"  # extra V drains on first tiles

NCORES = 8
R = int(os.environ.get("K_R", "256"))   # rows per tile
T = 2048 // R            # tiles per core
ROWS_PER_CORE = R * T    # 2048
D = 4096
TOTAL_ROWS = NCORES * ROWS_PER_CORE  # 16384

_F16 = np.float16


def _hadamard(n):
    H = np.array([[1.0]], dtype=np.float64)
    while H.shape[0] < n:
        H = np.block([[H, H], [H, -H]])
    return H


_H2 = _hadamard(2)
_H32 = _hadamard(32)
_H64 = _hadamard(64)


def _build_weights(inner_B, final_B):
    """w1/w3/w5 [128,4096] fp16 (32 lhsT blocks side by side), w2 [128,128]."""
    B0 = inner_B[0].astype(np.float64)
    B1 = inner_B[1].astype(np.float64)
    fB = final_B.astype(np.float64)

    C1 = np.einsum('vk,ukt->uvt', _H64, B1) / 64.0
    G = np.zeros((64, 64, 64))
    for u in range(64):
        for h in range(2):
            G[u][:, 32 * h:32 * h + 32] = _H64[:, 32 * h:32 * h + 32] @ fB[2 * u + h] / 64.0

    w1 = np.zeros((128, 32, 128))
    w3 = np.zeros((128, 32, 128))
    w5 = np.zeros((128, 32, 128))
    for m in range(32):
        for j in range(2):
            for jp in range(2):
                w1[j * 64:(j + 1) * 64, m, jp * 64:(jp + 1) * 64] = _H2[j, jp] * B0[2 * m + j]
                w3[j * 64:(j + 1) * 64, m, jp * 64:(jp + 1) * 64] = _H2[j, jp] * C1[2 * m + j]
        for jpp in range(2):
            w5[jpp * 64:(jpp + 1) * 64, m, jpp * 64:(jpp + 1) * 64] = G[2 * m + jpp]
    w1 = w1.reshape(128, 4096)
    w3 = w3.reshape(128, 4096)
    w5 = w5.reshape(128, 4096)
    w2 = np.kron(np.eye(4), _H32)      # partitions (X4, m)
    return (w1.astype(_F16), w2.astype(_F16), w3.astype(_F16), w5.astype(_F16))


def _pack_x(x):
    """x [..., 4096] fp32 -> list of per-core arrays [T, 128, 32*R] fp16."""
    xf = np.ascontiguousarray(x.reshape(-1, D))
    assert xf.shape[0] == TOTAL_ROWS
    x6 = xf.reshape(NCORES, T, R, 32, 2, 64)       # core,t,r,m,j,v
    x6 = x6.transpose(0, 1, 4, 5, 3, 2)            # core,t,j,v,m,r
    x6 = np.ascontiguousarray(x6).reshape(NCORES, T, 128, 32 * R)
    return [np.ascontiguousarray(x6[c]).astype(_F16) for c in range(NCORES)]


def _unpack_out(outs, orig_shape):
    """outs: list of per-core [T, 128, 32*R] fp16 -> [*orig_shape[:-1], 4096] fp32."""
    o = np.stack(outs, axis=0).astype(np.float32)  # [core, T, 128, 32R]
    o = o.reshape(NCORES, T, 2, 64, 32, R)         # core,t,j'',o,W,r
    o = o.transpose(0, 1, 5, 4, 2, 3)              # core,t,r,W,j'',o
    o = np.ascontiguousarray(o).reshape(TOTAL_ROWS, D)
    return o.reshape(*orig_shape[:-1], D)


# ---- bass program ----------------------------------------------------------
_PROGRAM = None


def _build_program():
    global _PROGRAM
    if _PROGRAM is not None:
        return _PROGRAM
    from contextlib import ExitStack
    import concourse.tile as tile
    from concourse import bacc, mybir

    f32 = mybir.dt.float32
    f16 = mybir.dt.float16
    i32 = mybir.dt.int32

    nc = bacc.Bacc()
    x_d = nc.declare_dram_parameter("x", [T, 128, 32 * R], f16, isOutput=False)
    w1_d = nc.declare_dram_parameter("w1", [128, 4096], f16, isOutput=False)
    w2_d = nc.declare_dram_parameter("w2", [128, 128], f16, isOutput=False)
    w3_d = nc.declare_dram_parameter("w3", [128, 4096], f16, isOutput=False)
    w5_d = nc.declare_dram_parameter("w5", [128, 4096], f16, isOutput=False)
    out_d = nc.declare_dram_parameter("out", [T, 128, 32 * R], f16, isOutput=True)

    C = 32 * R          # 8192 cols per tile
    HC = C // 2         # 4096: x half-tile DMA granularity

    with tile.TileContext(nc) as tc, ExitStack() as ctx:
        wpool = ctx.enter_context(tc.tile_pool(name="weights", bufs=1))
        xt_pool = ctx.enter_context(tc.tile_pool(name="xt", bufs=_XT_BUFS))
        yraw_pool = ctx.enter_context(tc.tile_pool(name="yraw", bufs=_YRAW_BUFS))
        yt_pool = ctx.enter_context(tc.tile_pool(name="yt", bufs=_YT_BUFS))
        out_pool = ctx.enter_context(tc.tile_pool(name="outp", bufs=_OUT_BUFS))
        psum = ctx.enter_context(tc.tile_pool(name="ps", bufs=4, space="PSUM"))

        w1_sb = wpool.tile([128, 4096], f16)
        w2_sb = wpool.tile([128, 128], f16)
        w3_sb = wpool.tile([128, 4096], f16)
        w5_sb = wpool.tile([128, 4096], f16)
        nc.sync.dma_start(w1_sb[:], w1_d[:])
        nc.sync.dma_start(w2_sb[:], w2_d[:])
        nc.sync.dma_start(w3_sb[:], w3_d[:])
        nc.sync.dma_start(w5_sb[:], w5_d[:])

        def w1_fn(b):
            return w1_sb[:, b * 128:(b + 1) * 128]

        sc = nc.scalar.copy
        vc = nc.vector.tensor_copy

        def pair_T(dst, src):
            """DVE 32x32 transpose of fp16 pairs (int32 view): swap partition
            low-5 bits with the 5-bit block index.  Layouts are (rh, blk, rl)
            with rl = 4 fp16 = 2 int32, so the 32-wide axis sits at 8-byte
            stride (full DVE rate)."""
            in_v = src[:].bitcast(i32).rearrange("p (rh m rl) -> p rh rl m", m=32, rl=2)
            out_v = dst[:].bitcast(i32).rearrange("p (rh tl rl) -> p rh rl tl", tl=32, rl=2)
            nc.vector.transpose(out_v, in_v)

        # 8 PSUM groups of 1024 fp32 (2 banks) per pass, pool bufs=4: four
        # matmul groups in flight, so TensorE + drains pipeline across passes
        # instead of forming one global drain-paced chain.

        def emit_blocked_pass(src_fn, w_fn, dst, vset=None):
            """Per-block lhsT pass (P1/P3): PSUM group g holds 4 blocks of 256
            cols in (i, rh, rl) order; drain writes dst in (rh, m, rl) layout
            via strided APs."""
            if vset is None:
                vset = _VD_BLOCKED
            dst_v = dst[:].rearrange("p (rh m rl) -> p rh m rl", m=32, rl=4)
            bpg = 1024 // R   # blocks per PSUM group
            for g in range(C // 1024):
                ps = psum.tile([128, 1024], f32, tag="ps")
                for i in range(bpg):
                    b = bpg * g + i
                    nc.tensor.matmul(
                        ps[:, i * R:(i + 1) * R],
                        w_fn(b),
                        src_fn(b),
                        start=True, stop=True,
                    )
                ps_v = ps[:].rearrange("p (m rh rl) -> p rh m rl", m=bpg, rl=4)
                eng = vc if g in vset else sc
                eng(dst_v[:, :, bpg * g:bpg * (g + 1), :], ps_v)

        def emit_fixed_pass(src, dst, w_sb, vset=None):
            """Fixed-weight pass (P2/P4): contiguous sweep with N=512
            matmuls, contiguous drains."""
            if vset is None:
                vset = _VD_FIXED
            for g in range(C // 1024):
                ps = psum.tile([128, 1024], f32, tag="ps")
                for i in range(2):
                    k = 2 * g + i
                    nc.tensor.matmul(
                        ps[:, i * 512:(i + 1) * 512],
                        w_sb[:],
                        src[:, k * 512:(k + 1) * 512],
                        start=True, stop=True,
                    )
                eng = vc if g in vset else sc
                eng(dst[:, g * 1024:(g + 1) * 1024], ps[:])

        def emit_p5(y4t, t):
            """P5: per-block W lhsT, rhs = 8B-run slices of y4t (rh, W, rl);
            PSUM (W-quad, rh, rl) drains contiguously to fp16 out (col = W*R+r).
            Out DMAs are batched: one 1MiB store per 4 drained groups."""
            y4t_v = y4t[:].rearrange("p (rh m rl) -> p m rh rl", m=32, rl=4)
            wpg = 1024 // R   # W-blocks per PSUM group
            OC = min(C, _OUT_COLS)
            for half in range(max(1, C // OC)):
                out_sb = out_pool.tile([128, OC], f16, tag="outp")
                for gq in range(OC // 1024):
                    g = (OC // 1024) * half + gq
                    ps = psum.tile([128, 1024], f32, tag="ps")
                    for i in range(wpg):
                        W = wpg * g + i
                        nc.tensor.matmul(
                            ps[:, i * R:(i + 1) * R],
                            w5_sb[:, W * 128:(W + 1) * 128],
                            y4t_v[:, W],
                            start=True, stop=True,
                        )
                    eng = vc if (g in _VD_P5 and (t % 2 == 1 or not _P5_ODD_ONLY)) else sc
                    eng(out_sb[:, gq * 1024:(gq + 1) * 1024], ps[:])
                nc.sync.dma_start(out_d[t][:, half * OC:(half + 1) * OC], out_sb[:])

        QC = C // _XQ       # cols per x-load chunk
        def load_x(t):
            xh = []
            for q in range(_XQ):
                xt = xt_pool.tile([128, QC], f16, tag="xt")
                nc.gpsimd.dma_start(xt[:], x_d[t][:, q * QC:(q + 1) * QC])
                xh.append(xt)
            return xh

        # ---- diagonal software pipeline -------------------------------
        S = [None] * T

        def stload(t):
            S[t] = {}
            S[t]['xh'] = load_x(t)

        def st0(t):
            S[t]['y1'] = yraw_pool.tile([128, C], f16, tag="yraw", name="y1")
            bph = QC // R   # blocks per x-load chunk
            vset = (1, 3, 5, 7) if (_RAMPV and t < 2) else None
            emit_blocked_pass(
                lambda b, xh=S[t]['xh'], bph=bph:
                    xh[b // bph][:, (b % bph) * R:(b % bph + 1) * R],
                w1_fn, S[t]['y1'], vset=vset)

        def st1(t):
            S[t]['y1t'] = yt_pool.tile([128, C], f16, tag="yt", name="y1t")
            pair_T(S[t]['y1t'], S[t]['y1'])

        def st2(t):
            S[t]['y2'] = yraw_pool.tile([128, C], f16, tag="yraw", name="y2")
            vset = (1, 3, 5, 7) if (_RAMPV and t < 1) else None
            emit_fixed_pass(S[t]['y1t'], S[t]['y2'], w2_sb, vset=vset)

        def st3(t):
            S[t]['y2t'] = yt_pool.tile([128, C], f16, tag="yt", name="y2t")
            pair_T(S[t]['y2t'], S[t]['y2'])

        def st4(t):
            S[t]['y3'] = yraw_pool.tile([128, C], f16, tag="yraw", name="y3")
            v = S[t]['y2t'][:].rearrange("p (rh m rl) -> p m rh rl", m=32, rl=4)
            emit_blocked_pass(
                lambda b, v=v: v[:, b],
                lambda b: w3_sb[:, b * 128:(b + 1) * 128], S[t]['y3'])

        def st5(t):
            S[t]['y3t'] = yt_pool.tile([128, C], f16, tag="yt", name="y3t")
            pair_T(S[t]['y3t'], S[t]['y3'])

        def st6(t):
            S[t]['y4'] = yraw_pool.tile([128, C], f16, tag="yraw", name="y4")
            emit_fixed_pass(S[t]['y3t'], S[t]['y4'], w2_sb)

        def st7(t):
            S[t]['y4t'] = yt_pool.tile([128, C], f16, tag="yt", name="y4t")
            pair_T(S[t]['y4t'], S[t]['y4'])

        def st8(t):
            emit_p5(S[t]['y4t'], t)
            S[t] = None

        stages = [stload, st0, st1, st2, st3, st4, st5, st6, st7, st8]
        NS = len(stages)
        for step in range(T + NS - 1):
            trange = range(T - 1, -1, -1) if _SHALLOW_FIRST else range(T)
            for t in trange:
                s = step - t
                if 0 <= s < NS:
                    stages[s](t)

    nc.finalize()
    _PROGRAM = nc
    return nc


_LAST_RESULTS = None


def build_for_profile(x, inner_B, final_B):
    """Return (nc, in_maps) for external profiling harnesses."""
    w1, w2, w3, w5 = _build_weights(np.asarray(inner_B), np.asarray(final_B))
    x_packed = _pack_x(np.asarray(x, dtype=np.float32))
    nc = _build_program()
    in_maps = [
        {"x": x_packed[c], "w1": w1, "w2": w2, "w3": w3, "w5": w5}
        for c in range(NCORES)
    ]
    return nc, in_maps


def kernel(x, inner_B, final_B, _trace=False):
    global _LAST_RESULTS
    from concourse.bass_utils import run_bass_kernel_spmd

    orig_shape = x.shape
    nc, in_maps = build_for_profile(x, inner_B, final_B)
    try:
        res = run_bass_kernel_spmd(nc, in_maps, list(range(NCORES)))
    except Exception:
        # transient NRT device errors have been observed; retry once
        res = run_bass_kernel_spmd(nc, in_maps, list(range(NCORES)))
    _LAST_RESULTS = res
    outs = [np.asarray(res.results[c]["out"]) for c in range(NCORES)]
    return _unpack_out(outs, orig_shape).astype(np.float32)
